# revision 51
# baseline (speedup 1.0000x reference)
"""GCN (3-layer message passing) distributed over 8 TRN2 NeuronCores.

Sharding: nodes split evenly across 8 cores (rows). Weights replicated.
Per layer: local matmul h = x @ W + b (node-major out via x^T-stationary
matmuls), then a CHUNKED AllGather of the h table: hloc row-range chunks
are AllGathered as soon as the matmul has produced them, so chunk c's
exchange (on TOPSP/SDMA silicon) overlaps the matmul for chunk c+1.
Each core then runs the local segment-sum over its incoming edges as
one-hot matmuls (edges tiled 128 at a time, PSUM-accumulated per
128-node destination window), gathering source rows straight from the
allgathered table; bias/relu fused in the epilogue. Gathers are batched
per (window-block, group) — one dma_gather covers ~6 windows' tiles —
so the per-call SWDGE fixed cost (~2us) is paid ~18x per layer instead
of ~100x; interior pad lanes gather row 0 and are zeroed by the
one-hot.

The allgathered table has NC*NPAD = 50176 rows, above the int16 gather
index ceiling, so each window's edges are split into two half-table
groups (chunks 0-1 / 2-3), gathered separately with rebased indices.
This also lets the first group's gathers start before the last chunks
arrive.

Layer 3's inclusion linear Wi is folded into W3 (segment_sum commutes
with right-matmul), so the last exchange is only 16 (padded 128) wide.

Everything data-dependent (edge tiling) is computed host-side in
preprocess(); all 8 cores run one SPMD graph whose shapes depend only
on those computed constants.
"""
import sys

sys.path.insert(0, "/opt/trn_rl_repo")

import numpy as np
import ml_dtypes

import concourse.bass as bass
import concourse.bacc as bacc
import concourse.mybir as mybir
import concourse.tile as tile
from concourse.bass_utils import run_bass_kernel_spmd

NC = 8
C = 2       # AllGather chunks per layer
NG = 2      # gather groups (half-tables) per window
BF16 = mybir.dt.bfloat16
F32 = mybir.dt.float32
I16 = mybir.dt.int16

last_exec_time_ns = None
last_results = None


def _wrap16(idx, ncols):
    """[n] int -> [128, n/16] int16 wrapped (idx i at [i%16, i//16]) and
    replicated to 128 partitions."""
    a = np.asarray(idx, np.int16).reshape(ncols, 16).T  # [16, n/16]
    return np.tile(a, (8, 1))


def preprocess(features, W1, b1, W2, b2, W3, b3, Wi, bi, src, dst,
               skip_pads=True):
    """Host-side sharding/setup. Returns (cfg dict, in_maps list).

    skip_pads: pad rows of each aggregation window get idx -1 and are
    skipped via num_idxs_reg (the matmul one-hot zeros them out; gather
    slots are pre-zeroed so stale lanes stay finite)."""
    N, K1t = features.shape  # 50000, 1433
    E = src.shape[0]
    assert N % NC == 0
    NLOC = N // NC
    MBLK = (NLOC + 127) // 128
    NPAD = MBLK * 128
    # chunk boundaries: chunk 0 ends on a 512-row (and window) boundary so
    # the pipelined mm emission can split exactly at the AG chunk edge
    CB = [0, (NPAD // C // 512) * 512, NPAD]   # [0, 3072, 6272]
    CR = [CB[i + 1] - CB[i] for i in range(C)]  # rows per chunk
    TBASE = [NC * CB[i] for i in range(C)]      # recv-table chunk base row
    GRs = [NC * CR[i] for i in range(C)]        # recv rows per gather group
    assert all(g <= 32768 for g in GRs)

    TW = [768, 512, 128]          # h-table widths (bf16, 256B-aligned)
    K = [1536, TW[0], TW[1]]      # matmul contraction dims (128-aligned)
    KB = [k // 128 for k in K]

    # ---- weights (fold Wi into W3), padded, bf16 ----
    W3f = (W3.astype(np.float64) @ Wi.astype(np.float64)).astype(np.float32)
    b3f = (b3.astype(np.float64) @ Wi.astype(np.float64)).astype(np.float32)

    def pad2(a, r, c):
        out = np.zeros((r, c), np.float32)
        out[: a.shape[0], : a.shape[1]] = a
        return out

    w1 = pad2(W1, K[0], TW[0]).astype(ml_dtypes.bfloat16)
    w2 = pad2(W2, K[1], TW[1]).astype(ml_dtypes.bfloat16)
    w3 = pad2(W3f, K[2], TW[2]).astype(ml_dtypes.bfloat16)
    b1p = np.tile(pad2(b1[None, :], 1, TW[0]), (128, 1))
    b2p = np.tile(pad2(b2[None, :], 1, TW[1]), (128, 1))
    b3p = np.tile(pad2(b3f[None, :], 1, TW[2]), (128, 1))
    bip = np.tile(pad2(bi[None, :], 1, TW[2]), (128, 1))

    # ---- per-core transposed features [K[0], NPAD] bf16 ----
    featTs = []
    for c in range(NC):
        ft = np.zeros((K[0], NPAD), np.float32)
        ft[:K1t, :NLOC] = features[c * NLOC : (c + 1) * NLOC].T
        featTs.append(ft.astype(ml_dtypes.bfloat16))

    # ---- graph structure ----
    src = np.asarray(src, np.int64)
    dst = np.asarray(dst, np.int64)
    owner = src // NLOC
    dcore = dst // NLOC

    # recv row of a source node (owner o, local row r), chunk-major:
    #   chunk c covers local rows [CB[c], CB[c+1]);
    #   row = TBASE[c] + o*CR[c] + (r - CB[c])
    # group g == chunk; gather indices are rebased by TBASE[g].
    per_core = []
    for d in range(NC):
        m = dcore == d
        es, ed = src[m], dst[m]
        eo = es // NLOC
        r = es - eo * NLOC
        ch = (r >= CB[1]).astype(np.int64)
        tbl = np.where(ch == 0,
                       eo * CR[0] + r,
                       TBASE[1] + eo * CR[1] + (r - CB[1]))
        grp = ch
        ldst = ed - d * NLOC
        win = ldst // 128
        rel = ldst % 128
        per_core.append((tbl, grp, win, rel))

    # ---- packed edge tiling ----
    # Windows grouped into blocks of WB. Per (block, group): all real edges
    # laid out contiguously (window-major), tile count = max over cores.
    # A tile may span window boundaries; the one-hot matmul runs per
    # (window, tile) SEGMENT with its own drel column (-1 on lanes that
    # belong to other windows). Trailing pad lanes are -1 and skipped via
    # per-sub-call counts, so only real edges cost gather descriptors.
    NW = MBLK
    WB = 3
    SUBT = 8  # max tiles per dma_gather call
    NB = (NW + WB - 1) // WB
    blocks = [(b * WB, min(WB, NW - b * WB)) for b in range(NB)]

    # per-core edge layout, sorted by (win, tbl) within each (b, g)
    counts = np.zeros((NC, NB, NG, WB), np.int64)  # edges per (core,b,g,w)
    for d in range(NC):
        tbl, grp, win, rel = per_core[d]
        for b, (w0, wc) in enumerate(blocks):
            for g in range(NG):
                for j in range(wc):
                    counts[d, b, g, j] = int(
                        ((win == w0 + j) & (grp == g)).sum())

    reg_edges = counts.sum(axis=3)            # [NC, NB, NG]
    BTg = np.zeros((NB, NG), np.int64)
    for b in range(NB):
        for g in range(NG):
            BTg[b, g] = max(1, int((reg_edges[:, b, g].max() + 127) // 128))
    T0 = np.zeros((NB, NG), np.int64)
    t = 0
    for b in range(NB):
        for g in range(NG):
            T0[b, g] = t
            t += int(BTg[b, g])
    TTOT = t
    T8 = ((TTOT + 7) // 8) * 8

    # segments: for (b,g,w): tile span = union over cores of
    # [start//128, (end-1)//128]; cores without edges there get all -1.
    # wsegs[w] = list of (g, t_local, seg_col) in group-then-tile order.
    starts = np.cumsum(counts, axis=3) - counts  # start offset per (c,b,g,w)
    ends = starts + counts
    wsegs = [[] for _ in range(NW)]
    seg_info = []  # (b, g, t_local, w)
    for b, (w0, wc) in enumerate(blocks):
        for g in range(NG):
            for j in range(wc):
                w = w0 + j
                lo, hi = None, None
                for d in range(NC):
                    if counts[d, b, g, j] > 0:
                        s0 = int(starts[d, b, g, j]) // 128
                        e0 = (int(ends[d, b, g, j]) - 1) // 128
                        lo = s0 if lo is None else min(lo, s0)
                        hi = e0 if hi is None else max(hi, e0)
                if lo is None:
                    continue
                for tl in range(lo, hi + 1):
                    col = len(seg_info)
                    seg_info.append((b, g, tl, w))
                    wsegs[w].append((g, tl, col))
    NSEG = len(seg_info)
    for w in range(NW):
        assert wsegs[w], f"window {w} has no edges"

    # sub-calls: each (b,g) region split into gathers of <= SUBT tiles
    subs = []  # per block: list of (g, s, st)
    for b in range(NB):
        bl = []
        for g in range(NG):
            tbg = int(BTg[b, g])
            for s in range(0, tbg, SUBT):
                bl.append((g, s, min(SUBT, tbg - s)))
        subs.append(bl)
    NSUB = sum(len(bl) for bl in subs)

    gidxs, dstrels, acnts = [], [], []
    for d in range(NC):
        tbl, grp, win, rel = per_core[d]
        order = np.lexsort((tbl, win))
        tbl, win, rel = tbl[order], win[order], rel[order]
        grp2 = grp[order]
        gi = np.full(T8 * 128, -1, np.int64)
        dr = np.full((128, NSEG), -1.0, np.float32)
        for b, (w0, wc) in enumerate(blocks):
            for g in range(NG):
                reg0 = int(T0[b, g]) * 128
                off = 0
                for j in range(wc):
                    mg = (win == w0 + j) & (grp2 == g)
                    n = int(mg.sum())
                    gi[reg0 + off : reg0 + off + n] = tbl[mg] - TBASE[g]
                    off += n
        # fill drel per segment
        for col, (b, g, tl, w) in enumerate(seg_info):
            w0, wc = blocks[b]
            j = w - w0
            s0 = int(starts[d, b, g, j])
            e0 = int(ends[d, b, g, j])
            lo = max(s0, tl * 128)
            hi = min(e0, (tl + 1) * 128)
            if hi <= lo:
                continue
            mg = (win == w) & (grp2 == g)
            rels = rel[mg]  # in tbl-sorted order == layout order
            lanes = np.arange(lo, hi) - tl * 128
            dr[lanes, col] = rels[lo - s0 : hi - s0]
        cn = []
        for b in range(NB):
            for (g, s, st) in subs[b]:
                cn.append(int(np.clip(int(reg_edges[d, b, g]) - s * 128,
                                      0, st * 128)))
        gidxs.append(_wrap16(gi, T8 * 8))
        dstrels.append(np.ascontiguousarray(dr))
        acnts.append(np.asarray(cn, np.int32).reshape(1, NSUB))

    iota = np.tile(np.arange(128, dtype=np.float32)[None, :], (128, 1))

    cfg = dict(NLOC=NLOC, NPAD=NPAD, MBLK=MBLK, TW=TW, K=K, KB=KB,
               CB=CB, TBASE=TBASE, GRs=GRs,
               blocks=blocks, subs=subs, NSUB=NSUB, NSEG=NSEG,
               wsegs=wsegs,
               T0=[[int(x) for x in row] for row in T0],
               BTg=[[int(x) for x in row] for row in BTg],
               T8=T8, OUT_W=16)

    in_maps = []
    for c in range(NC):
        in_maps.append({
            "featT": featTs[c],
            "w1": w1, "w2": w2, "w3": w3,
            "b1": b1p, "b2": b2p, "b3": b3p, "bi": bip,
            "gidx": gidxs[c], "drel": dstrels[c],
            "acnt": acnts[c], "iota": iota,
        })
    return cfg, in_maps


def build(cfg, nq=4, reps=1, queue_plan=None, collect_gathers=None):
    NLOC, NPAD, MBLK = cfg["NLOC"], cfg["NPAD"], cfg["MBLK"]
    TW, K, KB = cfg["TW"], cfg["K"], cfg["KB"]
    CB, TBASE, GRs = cfg["CB"], cfg["TBASE"], cfg["GRs"]
    T8, NSEG = cfg["T8"], cfg["NSEG"]
    blocks, T0, BTg = cfg["blocks"], cfg["T0"], cfg["BTg"]
    subs, NSUB, wsegs = cfg["subs"], cfg["NSUB"], cfg["wsegs"]
    OUT_W = cfg["OUT_W"]
    NW = MBLK
    NB = len(blocks)
    # sub-call flat index base per block
    sub_base = [0]
    for b in range(NB):
        sub_base.append(sub_base[-1] + len(subs[b]))

    AGP_BUFS = 4
    nc = bacc.Bacc("TRN2", target_bir_lowering=False, debug=False,
                   num_devices=NC, num_swdge_queues=nq)

    featT = nc.declare_dram_parameter("featT", [K[0], NPAD], BF16, isOutput=False)
    wts = [nc.declare_dram_parameter(f"w{l+1}", [K[l], TW[l]], BF16, isOutput=False)
           for l in range(3)]
    bs = [nc.declare_dram_parameter(f"b{l+1}", [128, TW[l]], F32, isOutput=False)
          for l in range(3)]
    bi = nc.declare_dram_parameter("bi", [128, TW[2]], F32, isOutput=False)
    gidx = nc.declare_dram_parameter("gidx", [128, T8 * 8], I16, isOutput=False)
    drel = nc.declare_dram_parameter("drel", [128, NSEG], F32, isOutput=False)
    acnt = nc.declare_dram_parameter("acnt", [1, NSUB], mybir.dt.int32,
                                     isOutput=False)
    iota = nc.declare_dram_parameter("iota", [128, 128], F32, isOutput=False)
    out = nc.declare_dram_parameter("out", [NLOC, OUT_W], F32, isOutput=True)

    hloc = [nc.dram_tensor(f"hloc{l}", [NPAD, TW[l]], BF16) for l in range(3)]
    recv = [nc.dram_tensor(f"recv{l}", [NC * NPAD, TW[l]], BF16,
                           addr_space="Shared") for l in range(3)]
    dumi = nc.dram_tensor("dumi", [128, 16], BF16)
    dumo = nc.dram_tensor("dumo", [NC * 128, 16], BF16, addr_space="Shared")
    xs = [None, nc.dram_tensor("x2", [NPAD, TW[0]], BF16),
          nc.dram_tensor("x3", [NPAD, TW[1]], BF16)]

    groups = [list(range(NC))]

    with tile.TileContext(nc) as tc:
        with (
            tc.tile_pool(name="wpool", bufs=1) as wpool,
            tc.tile_pool(name="bpool", bufs=1) as bpool,
            tc.tile_pool(name="ipool", bufs=1) as ipool,
            tc.tile_pool(name="xtp", bufs=2) as xtp,
            tc.tile_pool(name="mmpsum", bufs=2, space="PSUM") as mmpsum,
            tc.tile_pool(name="hbp", bufs=3) as hbp,
            tc.tile_pool(name="agp", bufs=AGP_BUFS) as agp,
            tc.tile_pool(name="ohp", bufs=6) as ohp,
            tc.tile_pool(name="apsum", bufs=2, space="PSUM") as apsum,
            tc.tile_pool(name="xop", bufs=3) as xop,
        ):
            # resident: indices, iota, dstrel
            gidx_t = ipool.tile([128, T8 * 8], I16, tag="gidx")
            nc.sync.dma_start(gidx_t[:], gidx[:])
            drel_t = ipool.tile([128, NSEG], F32, tag="drel")
            nc.sync.dma_start(drel_t[:], drel[:])
            iota_t = ipool.tile([128, 128], F32, tag="iota")
            nc.sync.dma_start(iota_t[:], iota[:])
            obuf = ipool.tile([128, NW, OUT_W], F32, tag="obuf")
            acnt_t = ipool.tile([1, NSUB], mybir.dt.int32, tag="acnt")
            nc.sync.dma_start(acnt_t[:], acnt[:])
            BTMAX = max(a + b for (a, b) in BTg)
            # zero the gather slots once so rows skipped by short gathers
            # (num_idxs_reg < num_idxs) read as finite values
            for _ in range(AGP_BUFS):
                zt = agp.tile([128, BTMAX, max(TW)], BF16, tag="ag")
                nc.vector.memset(zt[:], 0.0)
            nregs = [nc.gpsimd.alloc_register(name=f"nreg{i}")
                     for i in range(4)]
            gstate = {"gcall": 0}

            # layer-resident weights/biases
            wts_t, bts_t = {}, {}
            for l in range(3):
                wt = wpool.tile([128, KB[l], TW[l]], BF16, tag=f"w{l}")
                nc.sync.dma_start(
                    wt[:], wts[l].rearrange("(kb p) w -> p kb w", p=128))
                btl = bpool.tile([128, TW[l]], F32, tag=f"b{l}")
                nc.sync.dma_start(btl[:], bs[l][:])
                wts_t[l], bts_t[l] = wt, btl
            bit = bpool.tile([128, TW[2]], F32, tag="bi")
            nc.sync.dma_start(bit[:], bi[:])

            def emit_mm(l, r0, r1, part):
                """h[r0:r1] = x @ W + b for layer l."""
                wt, btl = wts_t[l], bts_t[l]
                nsl = [(s, min(s + 512, TW[l]))
                       for s in range(0, TW[l], 512)]
                sc = nc.enter_named_scope(f"mm{l}{part}", False)[0]
                for nr in range(r0, r1, 512):
                    rw = min(512, r1 - nr)
                    stripes = []
                    for kb in range(KB[l]):
                        st = xtp.tile([128, 512], BF16, tag=f"xt{kb}")
                        if l == 0:
                            nc.sync.dma_start(
                                st[:, :rw],
                                featT[kb * 128 : (kb + 1) * 128,
                                      nr : nr + rw])
                        else:
                            nc.sync.dma_start_transpose(
                                st[:, :rw],
                                xs[l][nr : nr + rw,
                                      kb * 128 : (kb + 1) * 128])
                        stripes.append(st)
                    for m in range(rw // 128):
                        ps = mmpsum.tile([128, TW[l]], F32, tag="mmps")
                        for kb in range(KB[l]):
                            for (s0, s1) in nsl:
                                nc.tensor.matmul(
                                    ps[:, s0:s1],
                                    stripes[kb][:, m * 128 : (m + 1) * 128],
                                    wt[:, kb, s0:s1],
                                    start=(kb == 0), stop=(kb == KB[l] - 1))
                        hb = hbp.tile([128, TW[l]], BF16, tag="hb")
                        nc.vector.tensor_tensor(
                            hb[:], ps[:], btl[:], op=mybir.AluOpType.add)
                        nc.sync.dma_start(
                            hloc[l][nr + m * 128 : nr + (m + 1) * 128, :],
                            hb[:])
                nc.leave_named_scope(f"mm{l}{part}", sc, False)

            def emit_ag(l, c):
                sc = nc.enter_named_scope(f"ag{l}c{c}", False)[0]
                nc.gpsimd.collective_compute(
                    "AllGather", mybir.AluOpType.bypass,
                    replica_groups=groups,
                    ins=[hloc[l][CB[c] : CB[c + 1], :]],
                    outs=[recv[l][NC * CB[c] : NC * CB[c + 1], :]])
                nc.leave_named_scope(f"ag{l}c{c}", sc, False)

            gts = {}  # block -> gather tile with g0 data (prefetched)

            def emit_sub(l, b, gt, g, s, st, scall):
                goff = (BTg[b][0] if g else 0) + s
                gcall = gstate["gcall"]
                nreg = nregs[gcall % 4]
                nc.gpsimd.reg_load(nreg, acnt_t[0:1, scall : scall + 1])
                gq = queue_plan[gcall] if queue_plan else gcall % nq
                t0 = T0[b][g] + s
                gi_ = nc.gpsimd.dma_gather(
                    gt[:, goff : goff + st, :],
                    recv[l][TBASE[g] : TBASE[g] + GRs[g]],
                    gidx_t[:, t0 * 8 : (t0 + st) * 8],
                    st * 128, nreg, TW[l], queue_num=gq)
                if collect_gathers is not None:
                    collect_gathers.append(gi_)
                gstate["gcall"] = gcall + 1

            def emit_prefetch(l, nblk):
                # group-0 gathers for the first blocks, emitted between the
                # two AG chunk triggers so they run during AG chunk 1
                # (the collective instruction blocks gpsimd to completion)
                for b in range(nblk):
                    btile = BTg[b][0] + BTg[b][1]
                    gt = agp.tile([128, btile, TW[l]], BF16, tag="ag")
                    gts[b] = gt
                    scall = sub_base[b]
                    for (g, s, st) in subs[b]:
                        if g == 0:
                            emit_sub(l, b, gt, g, s, st, scall)
                        scall += 1

            def emit_agg(l, b0, b1, part):
                nsl = [(s, min(s + 512, TW[l]))
                       for s in range(0, TW[l], 512)]
                sc = nc.enter_named_scope(f"agg{l}{part}", False)[0]
                for b in range(b0, b1):
                    w0, wc = blocks[b]
                    pre = b in gts
                    if pre:
                        gt = gts.pop(b)
                    else:
                        btile = BTg[b][0] + BTg[b][1]
                        gt = agp.tile([128, btile, TW[l]], BF16, tag="ag")
                    scall = sub_base[b]
                    for (g, s, st) in subs[b]:
                        if not (pre and g == 0):
                            emit_sub(l, b, gt, g, s, st, scall)
                        scall += 1
                    for w in range(w0, w0 + wc):
                        segsA = [s for s in wsegs[w] if s[0] == 0]
                        segsB = [s for s in wsegs[w] if s[0] == 1]

                        def run(ps, seglists):
                            total = sum(len(s) for s in seglists)
                            k = 0
                            for segs in seglists:
                                if not segs:
                                    continue
                                # one batched is_equal builds the whole
                                # group's one-hot stack (cols contiguous)
                                S = len(segs)
                                c0 = segs[0][2]
                                assert [s[2] for s in segs] == list(
                                    range(c0, c0 + S))
                                ohw = ohp.tile([128, S, 128], BF16,
                                               tag="oh")
                                nc.vector.tensor_tensor(
                                    ohw[:],
                                    drel_t[:, c0 : c0 + S].unsqueeze(2)
                                    .broadcast_to([128, S, 128]),
                                    iota_t[:].unsqueeze(1)
                                    .broadcast_to([128, S, 128]),
                                    op=mybir.AluOpType.is_equal)
                                for j, (g, tl, col) in enumerate(segs):
                                    gt_off = (BTg[b][0] if g else 0) + tl
                                    rhs = gt[:, gt_off, :]
                                    for (s0, s1) in nsl:
                                        nc.tensor.matmul(
                                            ps[:, s0:s1], ohw[:, j, :],
                                            rhs[:, s0:s1],
                                            start=(k == 0),
                                            stop=(k == total - 1))
                                    k += 1

                        psA = apsum.tile([128, TW[l]], F32, tag="apsA")
                        run(psA, [segsA, segsB])
                        # ---- epilogue ----
                        if l < 2:
                            xb = xop.tile([128, TW[l]], BF16, tag="xo")
                            nc.vector.tensor_scalar_max(
                                xb[:], psA[:], 0.0)
                            nc.sync.dma_start(
                                xs[l + 1][w * 128 : (w + 1) * 128, :],
                                xb[:])
                        else:
                            nc.vector.tensor_tensor(
                                obuf[:, w, :], psA[:, :OUT_W],
                                bit[:, :OUT_W],
                                op=mybir.AluOpType.add)
                            nc.vector.tensor_scalar_max(
                                obuf[:, w, :], obuf[:, w, :], 0.0)
                nc.leave_named_scope(f"agg{l}{part}", sc, False)

            # blocks covering the windows that feed mm chunk 0 (rows < CB[1])
            NBH = (CB[1] // 128 + blocks[0][1] - 1) // blocks[0][1]

            # tiny warm-up collective: absorbs the first-collective barrier
            # (cross-core rendezvous, can be 100s of us) under mm0
            nc.gpsimd.collective_compute(
                "AllGather", mybir.AluOpType.bypass, replica_groups=groups,
                ins=[dumi[:]], outs=[dumo[:]])

            # Sequential per-layer emission (baseline phase order): the
            # pipelined interleave measured SLOWER — mm chained behind the
            # previous layer's agg delays the next AG chunk by ~240us. The
            # packed tiling still cuts agg-phase Q7 emission ~35%.
            for _rep in range(reps):
                for l in range(3):
                    emit_mm(l, 0, CB[1], "a")
                    emit_ag(l, 0)
                    emit_mm(l, CB[1], NPAD, "b")
                    emit_ag(l, 1)
                    emit_agg(l, 0, NB, "a")
                # one batched store for the full windows, then the tail
                WFULL = NLOC // 128
                nc.sync.dma_start(
                    out[: WFULL * 128, :]
                    .rearrange("(w p) c -> p w c", p=128),
                    obuf[:, :WFULL, :])
                rows = NLOC - WFULL * 128
                if rows > 0:
                    nc.sync.dma_start(
                        out[WFULL * 128 :, :], obuf[:rows, WFULL, :])
    nc.finalize()
    return nc


DMASW0_IDX = 11  # PROC_NAME_TO_IDX["DMASW0"]


def build_lane_matched(cfg, reps=1):
    # queue spread is explicit now (round-robin per sub-call) — single pass.
    return build(cfg, nq=4, reps=reps)


def kernel(**inputs):
    global last_exec_time_ns, last_results
    inputs = {k: np.asarray(v) for k, v in inputs.items()}
    cfg, in_maps = preprocess(**inputs)
    nc = build_lane_matched(cfg)
    res = None
    # trace=True needs the axon NTFF hook; fall back to untraced runs, and
    # retry once more on transient device errors (NRT_EXEC_UNIT_UNRECOVERABLE).
    for attempt, trace in enumerate([True, False, False]):
        try:
            res = run_bass_kernel_spmd(
                nc, in_maps, core_ids=list(range(NC)), trace=trace)
            break
        except Exception:
            if attempt == 2:
                raise
            import time
            time.sleep(15)
    last_exec_time_ns = res.exec_time_ns
    last_results = res
    return np.concatenate([res.results[c]["out"] for c in range(NC)], axis=0)

